# revision 1
# baseline (speedup 1.0000x reference)
"""Cross-attention kernel for 8 Trainium2 NeuronCores (Bass/Tile, SPMD).

Reference computation (per batch b of 4):
    K_proj = K[b] @ Wk.T + bk            # [2048, 1024]
    V_proj = V[b] @ Wv.T + bv            # [2048, 1024]
    S      = Q[b] @ K_proj.T / 32        # [1024, 2048]
    P      = softmax(S, axis=-1)
    ctx    = P @ V_proj                  # [1024, 1024]
    out[b] = ctx @ Wo.T + bo             # [1024, 1024]

Sharding: 8 cores = 4 batches x 2 query-halves; no cross-core traffic.

Two exact algebraic reassociations shrink the per-core matmul work from
8.05 GMAC to 4.16 GMAC by contracting through the small (512-query)
side instead of materialising the [2048,1024] projections:

  scores:  Q @ (K@Wk.T + bk).T = (Q@Wk) @ K.T + (Q@bk) 1^T
           and a per-row constant cancels in softmax, so
           P = softmax((Q@Wk) @ K.T / 32) with bk dropped entirely.
  output:  ctx@Wo.T + bo = P@(V@Wv.T + bv)@Wo.T + bo
           = P@V @ (Wo@Wv).T + (Wo@bv + bo)     (P rows sum to 1)
           = (P@V) @ Wvo.T + c0.

Per-core pipeline (all layouts transposed, q always the free dim):
  qkT = (Q@Wk).T          [d2, q]    40960 PE cycles
  sT  = KT.T-contraction  [k,  q]    81920   (raw scores, exp along
                                              partitions, no max: |s|<=5)
  pT  = exp(sT/32)        [k,  q]
  Z   = ones^T @ pT       [*,  q]     8192   (row-sum bcast to 128 parts)
  cT  = (Pu@V).T          [d2, q]    81920   (two 5-bank PSUM passes)
  oT  = Wvo-contraction   [e,  q]    40960, then *1/Z + c0, DMA out.

All matmul operands are bf16 (host-side cast): same PE rate as f32r at
these tile sizes, half the DMA traffic.  Accumulation stays f32 in PSUM.

This container's walrus accepts at most ONE sync-wait command per
instruction.  Two local legalizations deal with that:
  * PatchedTileContext splits the final drain into one drain per
    outstanding proc.
  * split_multi_waits() hoists extra waits onto same-engine NoOps.
"""

import numpy as np

import concourse.bass as bass
import concourse.mybir as mybir
import concourse.tile as tile
from concourse.bass_utils import run_bass_kernel_spmd
from bass_rust import ScopedClock, VectorClock
from contextlib import ExitStack

F32 = mybir.dt.float32
BF16 = mybir.dt.bfloat16
EXP = mybir.ActivationFunctionType.Exp
CPY = mybir.ActivationFunctionType.Copy

B = 4
D1 = 1024
D2 = 1280
LK = 2048
LQ = 512          # queries per core
N_CORES = 8
SCALE = 1.0 / 32.0  # 1/sqrt(D1)

NT1 = D1 // 128   # 8  d1 tiles
NT2 = D2 // 128   # 10 d2 tiles
NTK = LK // 128   # 16 key tiles
NKB = LK // 512   # 4  key blocks of 512
H2 = 640          # half of d2 (5 tiles) for the two cT PSUM passes


class PatchedTileContext(tile.TileContext):
    """Split the end-of-context drain into single-wait drains."""

    def _drain_and_barrier(self, tick_clock, wait_clock):
        gc = tick_clock.global_clock
        n = len(gc)
        for i in range(n):
            t = gc[i]
            if t > 0:
                vec = [0] * n
                vec[i] = t
                d = self.nc.sync.drain()
                wait_clock.add_sem_waits(
                    d.ins, ScopedClock({None: VectorClock(vec)})
                )
        self.nc.all_engine_barrier()
        assert self.sems is not None
        popped = self.nc._tile_sem_poison_stack.pop()
        assert popped is self._sem_poison
        self.nc.clear_and_free_semaphores(list(self.sems.allocated().values()))
        self.nc.all_engine_barrier()


def split_multi_waits(nc, limit=1):
    """Hoist waits beyond `limit` onto same-engine NoOps placed directly
    before the offending instruction. Engine streams execute in order and
    Tile emits each stream in dependency-topological order, so this is
    semantics-preserving."""
    n_split = 0
    for fn in nc.m.functions:
        for blk in fn.blocks:
            il = blk.instructions
            newlist = []
            changed = False
            for inst in il:
                si = inst.sync_info
                ow = list(si.on_wait) if si is not None else []
                if len(ow) > limit:
                    for k, w in enumerate(ow[:-limit]):
                        nop = mybir.InstNoOp(
                            name=f"{inst.name}-ws{k}", ins=[], outs=[]
                        )
                        nop.engine = inst.engine
                        nop.sync_info = mybir.SyncInfo(on_wait=[w], on_update=[])
                        newlist.append(nop)
                        n_split += 1
                    inst.sync_info = mybir.SyncInfo(
                        on_wait=ow[-limit:], on_update=list(si.on_update)
                    )
                    changed = True
                newlist.append(inst)
            if changed:
                del il[:]
                il.extend(newlist)
    return n_split


def build_program(n_rounds=1):
    nc = bass.Bass()

    QT = nc.dram_tensor("QT", [D1, LQ], BF16, kind="ExternalInput")
    WkA = nc.dram_tensor("WkA", [D1, D2], BF16, kind="ExternalInput")
    KT = nc.dram_tensor("KT", [D2, LK], BF16, kind="ExternalInput")
    VN = nc.dram_tensor("VN", [LK, D2], BF16, kind="ExternalInput")
    WvoT = nc.dram_tensor("WvoT", [D2, D1], BF16, kind="ExternalInput")
    c0B = nc.dram_tensor("c0B", [128, NT1], F32, kind="ExternalInput")
    outT = nc.dram_tensor("outT", [D1, LQ], F32, kind="ExternalOutput")

    with PatchedTileContext(nc) as tc:
        es_stats = ExitStack()
        stats = es_stats.enter_context(tc.tile_pool(name="stats", bufs=1))
        ones_t = stats.tile([128, 128], BF16)
        nc.vector.memset(ones_t[:], 1.0)
        c0_t = stats.tile([128, NT1], F32)
        # c0_t's DMA is emitted inside emit_round, demoted behind the
        # phase-1 critical loads (first read is in phase 4).
        c0_loaded = [False]

        def emit_round(rnd):
            sfx = f"_{rnd}"
            es_w = ExitStack()        # wkA + qT (die after phase 1)
            es_qk = ExitStack()       # qkT (dies after sT)
            es_pt = ExitStack()       # pT + rb + cT (live to the end)
            es_ks = ExitStack()       # KT stream
            es_vn = ExitStack()       # VN stream
            es_wvo = ExitStack()      # WvoT tiles
            es_out = ExitStack()      # output staging
            es_pp1 = ExitStack()
            es_ppb = ExitStack()
            es_ppc = ExitStack()
            es_ppd = ExitStack()

            # Long-lived right-side tiles first (right pools release LIFO).
            # One tile per 512-column slice: Tile tracks dependencies at
            # tile granularity, and a single wide tile would make the first
            # downstream reader wait for ALL slice writers.
            pt = es_pt.enter_context(
                tc.tile_pool(name="pt" + sfx, bufs=1, side="right")
            )
            p_t = [pt.tile([128, LQ], BF16, tag=f"p{kt}", name=f"p{kt}" + sfx)
                   for kt in range(NTK)]
            c_t = [pt.tile([128, LQ], BF16, tag=f"c{t}", name=f"c{t}" + sfx)
                   for t in range(NT2)]
            rb = pt.tile([128, LQ], F32)         # broadcast 1/Z
            pqk = es_qk.enter_context(
                tc.tile_pool(name="pqk" + sfx, bufs=1, side="right")
            )
            qk_t = [pqk.tile([128, LQ], BF16, tag=f"qk{t}", name=f"qk{t}" + sfx)
                    for t in range(NT2)]
            # pwvo outlives pks (left pools release LIFO), so open it first.
            pwvo = es_wvo.enter_context(tc.tile_pool(name="pwvo" + sfx, bufs=1))
            pks = es_ks.enter_context(tc.tile_pool(name="pks" + sfx, bufs=2))

            # ---- phase 1: qkT = (Q @ Wk).T  [d2, q] ---------------------
            # pw rides the right stack above pqk so it can release first.
            pw = es_w.enter_context(
                tc.tile_pool(name="pw" + sfx, bufs=1, side="right")
            )
            wka_t, wkb_t = [], []
            qT = pw.tile([128, NT1 * LQ], BF16)
            for c in range(NT1):
                # Separate pass-A / pass-B column-half tiles (deps are
                # tile-granular) so the very first matmul only waits for
                # the 160 KB it reads.  Pass-B halves load after every
                # pass-A pair: DMAs serialize in emission order and pass A
                # consumes at DMA rate.
                wa = pw.tile([128, H2], BF16, tag=f"wka{c}", name=f"wka{c}" + sfx)
                wb = pw.tile([128, D2 - H2], BF16, tag=f"wkb{c}",
                             name=f"wkb{c}" + sfx)
                nc.sync.dma_start(wa[:], WkA[c * 128 : (c + 1) * 128, :H2])
                nc.sync.dma_start(
                    qT[:, c * LQ : (c + 1) * LQ], QT[c * 128 : (c + 1) * 128, :]
                )
                wka_t.append(wa)
                wkb_t.append(wb)
            for c in range(NT1):
                nc.sync.dma_start(
                    wkb_t[c][:], WkA[c * 128 : (c + 1) * 128, H2:]
                )
            if not c0_loaded[0]:
                nc.sync.dma_start(c0_t[:], c0B[:])
                c0_loaded[0] = True

            def load_ks_block(n):
                ks = [pks.tile([128, 512], BF16, tag=f"ks{f}",
                               name=f"ks{f}_{n}" + sfx) for f in range(NT2)]
                for f in range(NT2):
                    nc.sync.dma_start(
                        ks[f][:],
                        KT[f * 128 : (f + 1) * 128, n * 512 : (n + 1) * 512],
                    )
                return ks

            pp1 = es_pp1.enter_context(
                tc.tile_pool(name="pp1" + sfx, bufs=5, space="PSUM")
            )
            ks_blocks = {}
            # Pass A (t=0..4) is c-major so each matmul only needs the
            # (wkA[c], qT[c]) DMA pair that just landed.
            ps5 = [pp1.tile([128, LQ], F32, tag="pp1",
                            name=f"qkA_{t}" + sfx) for t in range(5)]
            for c in range(NT1):
                for t in range(5):
                    nc.tensor.matmul(
                        ps5[t][:],
                        wka_t[c][:, t * 128 : (t + 1) * 128],
                        qT[:, c * LQ : (c + 1) * LQ],
                        start=(c == 0),
                        stop=(c == NT1 - 1),
                    )
            for t in range(5):
                if t % 2 == 0:
                    nc.vector.tensor_copy(qk_t[t][:], ps5[t][:])
                else:
                    nc.scalar.activation(qk_t[t][:], ps5[t][:], CPY)
            # prefetch the first score block while pass B runs
            ks_blocks[0] = load_ks_block(0)
            # Pass B (t=5..9) is t-major (operands all resident by now) so
            # each tile's copy pipelines behind the next tile's matmuls.
            for t in range(5, NT2):
                ps = pp1.tile([128, LQ], F32, tag="pp1", name=f"qkB_{t}" + sfx)
                for c in range(NT1):
                    nc.tensor.matmul(
                        ps[:],
                        wkb_t[c][:, (t - 5) * 128 : (t - 4) * 128],
                        qT[:, c * LQ : (c + 1) * LQ],
                        start=(c == 0),
                        stop=(c == NT1 - 1),
                    )
                if t % 2 == 0:
                    nc.vector.tensor_copy(qk_t[t][:], ps[:])
                else:
                    nc.scalar.activation(qk_t[t][:], ps[:], CPY)
            es_pp1.close()
            es_w.close()

            # ---- phase 2: sT = qkT'KT, pT = exp(sT/32), Z = colsums -----
            wvo_t = []
            ppb = es_ppb.enter_context(
                tc.tile_pool(name="ppb" + sfx, bufs=3, space="PSUM")
            )
            ppz = es_ppb.enter_context(
                tc.tile_pool(name="ppz" + sfx, bufs=1, space="PSUM")
            )
            zps = ppz.tile([128, LQ], F32, tag="zps")

            def load_vn(half, kt):
                vn = pks.tile([128, H2], BF16, tag="vn",
                              name=f"vn{half}_{kt}" + sfx)
                nc.sync.dma_start(
                    vn[:],
                    VN[kt * 128 : (kt + 1) * 128, half * H2 : (half + 1) * H2],
                )
                return vn

            vn_pre = {}
            # Raw scores: |q.k|/32 <= ~5, so exp without max subtraction is
            # safe in f32 and matches the reference softmax to rounding.
            for n in range(NKB):
                ks = ks_blocks[n] if n in ks_blocks else load_ks_block(n)
                if n == NKB - 1:
                    # After the last score block: V-stream head first (read
                    # at phase 3 start), then the WvoT bulk (phase 4).
                    for kt in range(2):
                        vn_pre[(0, kt)] = load_vn(0, kt)
                    for f in range(NT2):
                        w = pwvo.tile([128, D1], BF16, tag=f"wvo{f}",
                                      name=f"wvo{f}" + sfx)
                        nc.sync.dma_start(w[:], WvoT[f * 128 : (f + 1) * 128, :])
                        wvo_t.append(w)
                for j in range(4):
                    kt = n * 4 + j
                    ps = ppb.tile([128, LQ], F32, tag="ppb")
                    for f in range(NT2):
                        nc.tensor.matmul(
                            ps[:],
                            ks[f][:, j * 128 : (j + 1) * 128],
                            qk_t[f][:],
                            start=(f == 0),
                            stop=(f == NT2 - 1),
                        )
                    nc.scalar.activation(
                        p_t[kt][:], ps[:], EXP, scale=SCALE
                    )
                    # Z accumulation, delayed one tile so the PE never waits
                    # on the exp of the tile it just produced.
                    if kt > 0:
                        nc.tensor.matmul(
                            zps[:],
                            ones_t[:],
                            p_t[kt - 1][:],
                            start=(kt == 1),
                            stop=False,
                        )
            nc.tensor.matmul(
                zps[:],
                ones_t[:],
                p_t[NTK - 1][:],
                start=False,
                stop=True,
            )
            nc.vector.reciprocal(rb[:], zps[:])
            es_qk.close()
            es_ppb.close()

            # ---- phase 3: cT = (Pu @ V).T  [d2, q] ----------------------
            # Ten [128,LQ] output tiles all accumulate over the full key
            # dim, so split d2 into two 5-bank PSUM passes; V is streamed
            # (and fetched) once per pass, half its columns each time.
            ppc = es_ppc.enter_context(
                tc.tile_pool(name="ppc" + sfx, bufs=5, space="PSUM")
            )
            for half in range(2):
                t0 = half * 5
                pc5 = [ppc.tile([128, LQ], F32, tag="ppc",
                                name=f"c{half}_{t}" + sfx) for t in range(5)]
                for kt in range(NTK):
                    vn = vn_pre.get((half, kt)) or load_vn(half, kt)
                    for t in range(5):
                        nc.tensor.matmul(
                            pc5[t][:],
                            vn[:, t * 128 : (t + 1) * 128],
                            p_t[kt][:],
                            start=(kt == 0),
                            stop=(kt == NTK - 1),
                        )
                for t in range(5):
                    if t % 2 == 0:
                        nc.vector.tensor_copy(c_t[t0 + t][:], pc5[t][:])
                    else:
                        nc.scalar.activation(c_t[t0 + t][:], pc5[t][:], CPY)
            es_ks.close()
            es_ppc.close()

            # ---- phase 4: oT = (Wvo-contract cT) * rb + c0  [e, q] ------
            # 3 bufs: ppc's 5 banks stay allocated until its copies drain,
            # and 5+4 would make the first oT psum wait on a ppc free.
            ppd = es_ppd.enter_context(
                tc.tile_pool(name="ppd" + sfx, bufs=3, space="PSUM")
            )
            posb = es_out.enter_context(tc.tile_pool(name="posb" + sfx, bufs=2))
            for et in range(NT1):
                ps = ppd.tile([128, LQ], F32, tag="ppd")
                for t in range(NT2):
                    nc.tensor.matmul(
                        ps[:],
                        wvo_t[t][:, et * 128 : (et + 1) * 128],
                        c_t[t][:],
                        start=(t == 0),
                        stop=(t == NT2 - 1),
                    )
                if et < NT1 - 1:
                    ob = posb.tile([128, LQ], F32, tag="osb")
                    nc.vector.tensor_mul(ob[:], ps[:], rb[:])
                    nc.vector.tensor_scalar_add(
                        ob[:], ob[:], c0_t[:, et : et + 1]
                    )
                    nc.sync.dma_start(outT[et * 128 : (et + 1) * 128, :], ob[:])
                else:
                    # last tile: column halves pipeline DVE against DMA so
                    # less of the fixup trails the final matmul
                    for hh in range(2):
                        obh = posb.tile([128, LQ // 2], F32, tag=f"osbh{hh}")
                        sl = slice(hh * (LQ // 2), (hh + 1) * (LQ // 2))
                        nc.vector.tensor_mul(obh[:], ps[:, sl], rb[:, sl])
                        nc.vector.tensor_scalar_add(
                            obh[:], obh[:], c0_t[:, et : et + 1]
                        )
                        nc.sync.dma_start(
                            outT[et * 128 : (et + 1) * 128, sl], obh[:]
                        )
            es_out.close()
            es_wvo.close()
            es_pt.close()
            es_ppd.close()

        for rnd in range(n_rounds):
            emit_round(rnd)
        es_stats.close()

    split_multi_waits(nc)
    return nc


_PROGRAM = None


def _get_program():
    global _PROGRAM
    if _PROGRAM is None:
        _PROGRAM = build_program()
    return _PROGRAM


def build_in_maps(inputs):
    bf16 = mybir.dt.np(BF16)
    Q = np.asarray(inputs["Q"], dtype=np.float32)
    K = np.asarray(inputs["K"], dtype=np.float32)
    V = np.asarray(inputs["V"], dtype=np.float32)
    Wk = np.asarray(inputs["Wk"], dtype=np.float32)
    Wv = np.asarray(inputs["Wv"], dtype=np.float32)
    Wo = np.asarray(inputs["Wo"], dtype=np.float32)
    bv = np.asarray(inputs["bv"], dtype=np.float32)
    bo = np.asarray(inputs["bo"], dtype=np.float32)
    # bk drops out: it shifts every logit of a softmax row by the same
    # constant.

    Wvo = Wo @ Wv                                   # [D1, D2]
    c0 = Wo @ bv + bo                               # [D1]
    WkA_h = np.ascontiguousarray(Wk).astype(bf16)       # [D1, D2]
    WvoT_h = np.ascontiguousarray(Wvo.T).astype(bf16)   # [D2, D1]
    c0B_h = np.ascontiguousarray(c0.reshape(NT1, 128).T).astype(np.float32)
    KT_h = [np.ascontiguousarray(K[b].T).astype(bf16) for b in range(B)]
    VN_h = [np.ascontiguousarray(V[b]).astype(bf16) for b in range(B)]

    in_maps = []
    for c in range(N_CORES):
        b, h = divmod(c, 2)
        in_maps.append(
            {
                "QT": np.ascontiguousarray(
                    Q[b, h * LQ : (h + 1) * LQ, :].T
                ).astype(bf16),
                "WkA": WkA_h,
                "KT": KT_h[b],
                "VN": VN_h[b],
                "WvoT": WvoT_h,
                "c0B": c0B_h,
            }
        )
    return in_maps


def assemble_output(results):
    out = np.empty((B, 2 * LQ, D1), dtype=np.float32)
    for c in range(N_CORES):
        b, h = divmod(c, 2)
        out[b, h * LQ : (h + 1) * LQ, :] = results[c]["outT"].T
    return out


def kernel(Q, K, V, Wk, bk, Wv, bv, Wo, bo):
    inputs = dict(Q=Q, K=K, V=V, Wk=Wk, bk=bk, Wv=Wv, bv=bv, Wo=Wo, bo=bo)
    nc = _get_program()
    in_maps = build_in_maps(inputs)
    res = run_bass_kernel_spmd(nc, in_maps, list(range(N_CORES)))
    return assemble_output(res.results)



# revision 12
# speedup vs baseline: 1.1591x; 1.1591x over previous
"""Cross-attention kernel for 8 Trainium2 NeuronCores (Bass/Tile, SPMD).

Reference computation (per batch b of 4):
    K_proj = K[b] @ Wk.T + bk            # [2048, 1024]
    V_proj = V[b] @ Wv.T + bv            # [2048, 1024]
    S      = Q[b] @ K_proj.T / 32        # [1024, 2048]
    P      = softmax(S, axis=-1)
    ctx    = P @ V_proj                  # [1024, 1024]
    out[b] = ctx @ Wo.T + bo             # [1024, 1024]

Sharding: 8 cores = 4 batches x 2 query-halves; no cross-core traffic.

Algebraic reassociations (exact) shrink per-core matmul work to 4.16
GMAC by contracting through the small (512-query) side:
  P   = softmax((Q@Wk) @ K.T / 32)            (bk cancels in softmax)
  out = (P@V) @ (Wo@Wv).T / Z + (Wo@bv + bo)  (P rows sum to 1)

All four big matmuls run as fp8e4 DoubleRow (256-deep contraction at
0.5 PE cycles per output row = 4x the bf16 MAC rate).  Accuracy is kept
near-bf16 with hi/lo residual splitting: every operand X is carried as
X_hi = fp8(X), X_lo = fp8(X - X_hi), and each product uses three terms
  X Y ~ Xh Yh + Xl Yh + Xh Yl        (the lo*lo term is ~2^-8, dropped)
at 3/4 of the bf16 PE cost.  Weights/K/V/Q are split on the host; the
on-chip intermediates (qk, p, c) are split with act/DVE copy+sub pairs
that pipeline behind the PE.

Per-core pipeline (transposed layouts, q always the free dim):
  qkT = (Q@Wk).T          [d2, q]    30720 PE cycles
  sT  = KT-contraction    [k,  q]    61440  (raw scores, exp, no max:
                                             |s/32| <= ~5)
  Z   = ones^T (ph + pl)  [*,  q]     4096  (DoubleRow over hi/lo pair)
  cT  = (P@V).T           [d2, q]    61440
  oT  = Wvo-contraction   [e,  q]    30720, then *1/Z + c0, DMA out.

This container's walrus accepts at most ONE sync-wait command per
instruction; PatchedTileContext + split_multi_waits legalize that.
"""

import numpy as np

import concourse.bass as bass
import concourse.mybir as mybir
import concourse.tile as tile
from concourse.bass_utils import run_bass_kernel_spmd
from bass_rust import ScopedClock, VectorClock
from contextlib import ExitStack

F32 = mybir.dt.float32
BF16 = mybir.dt.bfloat16
FP8 = mybir.dt.float8e4
NP_FP8 = mybir.dt.np(FP8)
EXP = mybir.ActivationFunctionType.Exp
CPY = mybir.ActivationFunctionType.Copy
DR = mybir.MatmulPerfMode.DoubleRow

B = 4
D1 = 1024
D2 = 1280
LK = 2048
LQ = 512          # queries per core
N_CORES = 8
# Wk is carried as 32*Wk and Wvo as 64*Wvo: their raw elements (std
# ~0.013-0.02) sit below fp8e4m3's min normal 2^-6 and would quantize as
# subnormals.  The 32x folds into the exp scale (scores arrive 32x big),
# the 64x folds into Z via the ones=64 colsum weights.
SCALE = 1.0 / 1024.0  # (1/sqrt(D1)) / 32
# p = exp(s/32 - ABIAS): the constant attenuation cancels exactly through
# Z but keeps the unnormalized P@V accumulation inside fp8e4 range (+-240).
ABIAS = 1.25

NT1 = D1 // 128   # 8  d1 tiles (phase-4 output tiles)
NT2 = D2 // 128   # 10 d2 tiles
NTK = LK // 128   # 16 key tiles
ND1 = D1 // 256   # 4  d1 double-tiles
ND2 = D2 // 256   # 5  d2 double-tiles
NDK = LK // 256   # 8  key double-tiles
NKB = LK // 512   # 4  key blocks of 512
H2 = 640          # half of d2 (5 tiles): phase-1 groups / phase-3 passes


class PatchedTileContext(tile.TileContext):
    """Split the end-of-context drain into single-wait drains."""

    def _drain_and_barrier(self, tick_clock, wait_clock):
        gc = tick_clock.global_clock
        n = len(gc)
        for i in range(n):
            t = gc[i]
            if t > 0:
                vec = [0] * n
                vec[i] = t
                d = self.nc.sync.drain()
                wait_clock.add_sem_waits(
                    d.ins, ScopedClock({None: VectorClock(vec)})
                )
        self.nc.all_engine_barrier()
        assert self.sems is not None
        popped = self.nc._tile_sem_poison_stack.pop()
        assert popped is self._sem_poison
        self.nc.clear_and_free_semaphores(list(self.sems.allocated().values()))
        self.nc.all_engine_barrier()


def split_multi_waits(nc, limit=1):
    """Hoist waits beyond `limit` onto same-engine NoOps placed directly
    before the offending instruction. Engine streams execute in order and
    Tile emits each stream in dependency-topological order, so this is
    semantics-preserving."""
    n_split = 0
    for fn in nc.m.functions:
        for blk in fn.blocks:
            il = blk.instructions
            newlist = []
            changed = False
            for inst in il:
                si = inst.sync_info
                ow = list(si.on_wait) if si is not None else []
                if len(ow) > limit:
                    for k, w in enumerate(ow[:-limit]):
                        nop = mybir.InstNoOp(
                            name=f"{inst.name}-ws{k}", ins=[], outs=[]
                        )
                        nop.engine = inst.engine
                        nop.sync_info = mybir.SyncInfo(on_wait=[w], on_update=[])
                        newlist.append(nop)
                        n_split += 1
                    inst.sync_info = mybir.SyncInfo(
                        on_wait=ow[-limit:], on_update=list(si.on_update)
                    )
                    changed = True
                newlist.append(inst)
            if changed:
                del il[:]
                il.extend(newlist)
    return n_split


def build_program(n_rounds=1):
    nc = bass.Bass()

    QH = nc.dram_tensor("QH", [128, ND1, 2, LQ], FP8, kind="ExternalInput")
    QL = nc.dram_tensor("QL", [128, ND1, 2, LQ], FP8, kind="ExternalInput")
    WKH = nc.dram_tensor("WKH", [128, ND1, 2, D2], FP8, kind="ExternalInput")
    WKL = nc.dram_tensor("WKL", [128, ND1, 2, D2], FP8, kind="ExternalInput")
    KH = nc.dram_tensor("KH", [128, NKB, ND2, 2, 512], FP8, kind="ExternalInput")
    KL = nc.dram_tensor("KL", [128, NKB, ND2, 2, 512], FP8, kind="ExternalInput")
    VH = nc.dram_tensor("VH", [128, 2, NDK, 2, H2], FP8, kind="ExternalInput")
    VL = nc.dram_tensor("VL", [128, 2, NDK, 2, H2], FP8, kind="ExternalInput")
    WVH = nc.dram_tensor("WVH", [128, ND2, 2, D1], FP8, kind="ExternalInput")
    WVL = nc.dram_tensor("WVL", [128, ND2, 2, D1], FP8, kind="ExternalInput")
    c0B = nc.dram_tensor("c0B", [128, NT1], F32, kind="ExternalInput")
    outT = nc.dram_tensor("outT", [D1, LQ], F32, kind="ExternalOutput")

    with PatchedTileContext(nc) as tc:
        es_stats = ExitStack()
        stats = es_stats.enter_context(tc.tile_pool(name="stats", bufs=1))
        ones_t = stats.tile([128, 2, 128], FP8)
        nc.vector.memset(ones_t[:], 64.0)  # folds Wvo's 64x into Z
        abias_t = stats.tile([128, 1], F32)
        nc.vector.memset(abias_t[:], -ABIAS)
        c0_t = stats.tile([128, NT1], F32)
        c0_loaded = [False]

        def emit_round(rnd):
            sfx = f"_{rnd}"
            es_w = ExitStack()        # q + wk tiles (die after phase 1)
            es_qk = ExitStack()       # qk doubles (die after phase 2)
            es_pt = ExitStack()       # p + c doubles + rb (live to the end)
            es_ks = ExitStack()       # K blocks + V halves stream
            es_wvo = ExitStack()      # Wvo tiles
            es_exp = ExitStack()      # exp f32 staging
            es_out = ExitStack()      # output staging
            es_pp1 = ExitStack()
            es_ppb = ExitStack()
            es_ppz = ExitStack()
            es_ppc = ExitStack()
            es_ppd = ExitStack()

            # Long-lived right-side tiles first (right pools release LIFO).
            # Separate tiles per slice: Tile tracks deps at tile granularity.
            pt = es_pt.enter_context(
                tc.tile_pool(name="pt" + sfx, bufs=1, side="right")
            )
            ph_d = [pt.tile([128, 2, LQ], FP8, tag=f"ph{k}", name=f"ph{k}" + sfx)
                    for k in range(NDK)]
            pl_d = [pt.tile([128, 2, LQ], FP8, tag=f"pl{k}", name=f"pl{k}" + sfx)
                    for k in range(NDK)]
            ch_d = [pt.tile([128, 2, LQ], FP8, tag=f"ch{t}", name=f"ch{t}" + sfx)
                    for t in range(ND2)]
            cl_d = [pt.tile([128, 2, LQ], FP8, tag=f"cl{t}", name=f"cl{t}" + sfx)
                    for t in range(ND2)]
            rb = pt.tile([128, LQ], F32)         # broadcast 1/Z
            pqk = es_qk.enter_context(
                tc.tile_pool(name="pqk" + sfx, bufs=1, side="right")
            )
            qh_d = [pqk.tile([128, 2, LQ], FP8, tag=f"qh{f}", name=f"qh{f}" + sfx)
                    for f in range(ND2)]
            ql_d = [pqk.tile([128, 2, LQ], FP8, tag=f"ql{f}", name=f"ql{f}" + sfx)
                    for f in range(ND2)]
            # pwvo outlives pks (left pools release LIFO), so open it first.
            pwvo = es_wvo.enter_context(tc.tile_pool(name="pwvo" + sfx, bufs=1))
            pexp = es_exp.enter_context(tc.tile_pool(name="pexp" + sfx, bufs=3))
            pks = es_ks.enter_context(tc.tile_pool(name="pks" + sfx, bufs=2))

            # ---- phase 1: qkT = (Q @ Wk).T  [d2, q] ---------------------
            pw = es_w.enter_context(
                tc.tile_pool(name="pw" + sfx, bufs=1, side="right")
            )
            q_hi = pw.tile([128, ND1, 2, LQ], FP8, tag="qhi", name="qhi" + sfx)
            q_lo = pw.tile([128, ND1, 2, LQ], FP8, tag="qlo", name="qlo" + sfx)
            # wk[hi/lo][c][g]: [128, 2, H2] tiles; group g covers d2 columns
            # [g*H2, (g+1)*H2).
            wkh = [[pw.tile([128, 2, H2], FP8, tag=f"wkh{c}{g}",
                            name=f"wkh{c}{g}" + sfx) for g in range(2)]
                   for c in range(ND1)]
            wkl = [[pw.tile([128, 2, H2], FP8, tag=f"wkl{c}{g}",
                            name=f"wkl{c}{g}" + sfx) for g in range(2)]
                   for c in range(ND1)]
            # DMA order = queue order: everything phase-1 pass G0 needs
            # first, then G1, then the K/V/Wvo stream.
            nc.sync.dma_start(q_hi[:], QH[:])
            for c in range(ND1):
                nc.sync.dma_start(wkh[c][0][:], WKH[:, c, :, :H2])
            for c in range(ND1):
                nc.sync.dma_start(wkl[c][0][:], WKL[:, c, :, :H2])
            nc.sync.dma_start(q_lo[:], QL[:])
            for c in range(ND1):
                nc.sync.dma_start(wkh[c][1][:], WKH[:, c, :, H2:])
            for c in range(ND1):
                nc.sync.dma_start(wkl[c][1][:], WKL[:, c, :, H2:])
            if not c0_loaded[0]:
                nc.sync.dma_start(c0_t[:], c0B[:])
                c0_loaded[0] = True

            def load_k_block(n):
                kh = pks.tile([128, ND2, 2, 512], FP8, tag="ksh",
                              name=f"ksh{n}" + sfx)
                kl = pks.tile([128, ND2, 2, 512], FP8, tag="ksl",
                              name=f"ksl{n}" + sfx)
                nc.sync.dma_start(kh[:], KH[:, n])
                nc.sync.dma_start(kl[:], KL[:, n])
                return kh, kl

            def load_v_half(h):
                vh = pks.tile([128, NDK, 2, H2], FP8, tag="vh",
                              name=f"vh{h}" + sfx)
                nc.sync.dma_start(vh[:], VH[:, h])
                return vh

            def load_v_half_lo(h):
                vl = pks.tile([128, NDK, 2, H2], FP8, tag="vl",
                              name=f"vl{h}" + sfx)
                nc.sync.dma_start(vl[:], VL[:, h])
                return vl

            pp1 = es_pp1.enter_context(
                tc.tile_pool(name="pp1" + sfx, bufs=5, space="PSUM")
            )
            # qk split targets: singles s = 5g + t -> double s//2, half s%2
            def qk_split(ps, s, parity):
                dst_h = qh_d[s // 2][:, s % 2, :]
                dst_l = ql_d[s // 2][:, s % 2, :]
                if parity % 2 == 0:
                    nc.scalar.activation(dst_h, ps[:], CPY)
                else:
                    nc.vector.tensor_copy(dst_h, ps[:])
                nc.vector.tensor_sub(dst_l, ps[:], dst_h)

            for g in range(2):
                ps5 = [pp1.tile([128, LQ], F32, tag="pp1",
                                name=f"qk{g}_{t}" + sfx) for t in range(5)]
                steps = (
                    [(wkh[c][g], q_hi[:, c]) for c in range(ND1)]
                    + [(wkl[c][g], q_hi[:, c]) for c in range(ND1)]
                    + [(wkh[c][g], q_lo[:, c]) for c in range(ND1)]
                )
                n_steps = len(steps)
                for i, (w, q) in enumerate(steps):
                    for t in range(5):
                        nc.tensor.matmul(
                            ps5[t][:],
                            w[:, :, t * 128 : (t + 1) * 128],
                            q,
                            start=(i == 0),
                            stop=(i == n_steps - 1),
                            perf_mode=DR,
                        )
                for t in range(5):
                    qk_split(ps5[t], 5 * g + t, t)
                if g == 0:
                    # prefetch the first score block while G1 runs
                    kb0 = load_k_block(0)
            es_pp1.close()
            es_w.close()

            # ---- phase 2: sT = qkT'KT, pT = exp(sT/32) hi/lo, Z ---------
            ppb = es_ppb.enter_context(
                tc.tile_pool(name="ppb" + sfx, bufs=3, space="PSUM")
            )
            ppz = es_ppz.enter_context(
                tc.tile_pool(name="ppz" + sfx, bufs=1, space="PSUM",
                             side="right")
            )
            zps = ppz.tile([128, LQ], F32, tag="zps")

            def emit_z(ktd):
                # Z accumulates ones^T (p_hi + p_lo) over all key doubles.
                nc.tensor.matmul(
                    zps[:], ones_t[:], ph_d[ktd][:],
                    start=(ktd == 0), stop=False, perf_mode=DR,
                )
                nc.tensor.matmul(
                    zps[:], ones_t[:], pl_d[ktd][:],
                    start=False, stop=(ktd == NDK - 1), perf_mode=DR,
                )

            blocks = {0: kb0}
            v_half = {}
            wv_t = {}
            for n in range(NKB):
                kh, kl = blocks.pop(n)
                # stream the next loads down the single DMA queue
                if n < NKB - 1:
                    blocks[n + 1] = load_k_block(n + 1)
                if n == 1:
                    v_half[(0, "h")] = load_v_half(0)
                    v_half[(0, "l")] = load_v_half_lo(0)
                if n == 3:
                    wv_t["h"] = pwvo.tile([128, ND2, 2, D1], FP8, tag="wvh",
                                          name="wvh" + sfx)
                    nc.sync.dma_start(wv_t["h"][:], WVH[:])
                    v_half[(1, "h")] = load_v_half(1)
                    v_half[(1, "l")] = load_v_half_lo(1)
                    wv_t["l"] = pwvo.tile([128, ND2, 2, D1], FP8, tag="wvl",
                                          name="wvl" + sfx)
                    nc.sync.dma_start(wv_t["l"][:], WVL[:])
                for j in range(4):
                    kt = n * 4 + j
                    ps = ppb.tile([128, LQ], F32, tag="ppb")
                    terms = [(kh, qh_d), (kl, qh_d), (kh, ql_d)]
                    for ti, (kx, qx) in enumerate(terms):
                        for f in range(ND2):
                            nc.tensor.matmul(
                                ps[:],
                                kx[:, f, :, j * 128 : (j + 1) * 128],
                                qx[f][:],
                                start=(ti == 0 and f == 0),
                                stop=(ti == 2 and f == ND2 - 1),
                                perf_mode=DR,
                            )
                    # raw scores: |s|/32 <= ~5, exp without max is safe
                    pe_t = pexp.tile([128, LQ], F32, tag="pe")
                    nc.scalar.activation(
                        pe_t[:], ps[:], EXP, scale=SCALE, bias=abias_t[:]
                    )
                    dst_h = ph_d[kt // 2][:, kt % 2, :]
                    dst_l = pl_d[kt // 2][:, kt % 2, :]
                    if kt % 2 == 0:
                        nc.scalar.activation(dst_h, pe_t[:], CPY)
                    else:
                        nc.vector.tensor_copy(dst_h, pe_t[:])
                    nc.vector.tensor_sub(dst_l, pe_t[:], dst_h)
                    # Z lagged 2 key-tiles behind the exp/split pipeline so
                    # the PE never waits on DVE.
                    if kt >= 2 and kt % 2 == 0:
                        emit_z(kt // 2 - 1)
            es_qk.close()
            es_ppb.close()

            # ---- phase 3: cT = (Pu @ V).T  [d2, q] ----------------------
            # t-major per half so output singles complete staggered and
            # their hi/lo splits pipeline behind the PE.
            ppc = es_ppc.enter_context(
                tc.tile_pool(name="ppc" + sfx, bufs=5, space="PSUM")
            )
            for h in range(2):
                vh = v_half[(h, "h")]
                vl = v_half[(h, "l")]
                for t in range(5):
                    pc = ppc.tile([128, LQ], F32, tag="ppc",
                                  name=f"c{h}_{t}" + sfx)
                    terms = [(vh, ph_d), (vl, ph_d), (vh, pl_d)]
                    for ti, (vx, px) in enumerate(terms):
                        for ktd in range(NDK):
                            nc.tensor.matmul(
                                pc[:],
                                vx[:, ktd, :, t * 128 : (t + 1) * 128],
                                px[ktd][:],
                                start=(ti == 0 and ktd == 0),
                                stop=(ti == 2 and ktd == NDK - 1),
                                perf_mode=DR,
                            )
                    s = 5 * h + t
                    dst_h = ch_d[s // 2][:, s % 2, :]
                    dst_l = cl_d[s // 2][:, s % 2, :]
                    if s % 2 == 0:
                        nc.scalar.activation(dst_h, pc[:], CPY)
                    else:
                        nc.vector.tensor_copy(dst_h, pc[:])
                    nc.vector.tensor_sub(dst_l, pc[:], dst_h)
                    if h == 0 and t == 0:
                        # trailing Z pair; p splits land ~1us into phase 3
                        emit_z(NDK - 1)
                        nc.vector.reciprocal(rb[:], zps[:])
                        es_ppz.close()
            es_ks.close()
            es_ppc.close()

            # ---- phase 4: oT = (Wvo-contract cT) * rb + c0  [e, q] ------
            ppd = es_ppd.enter_context(
                tc.tile_pool(name="ppd" + sfx, bufs=3, space="PSUM")
            )
            posb = es_out.enter_context(tc.tile_pool(name="posb" + sfx, bufs=2))
            wvh_t = wv_t["h"]
            wvl_t = wv_t["l"]
            for et in range(NT1):
                ps = ppd.tile([128, LQ], F32, tag="ppd")
                terms = [(wvh_t, ch_d), (wvl_t, ch_d), (wvh_t, cl_d)]
                for ti, (wx, cx) in enumerate(terms):
                    for td in range(ND2):
                        nc.tensor.matmul(
                            ps[:],
                            wx[:, td, :, et * 128 : (et + 1) * 128],
                            cx[td][:],
                            start=(ti == 0 and td == 0),
                            stop=(ti == 2 and td == ND2 - 1),
                            perf_mode=DR,
                        )
                if et < NT1 - 1:
                    ob = posb.tile([128, LQ], F32, tag="osb")
                    nc.vector.tensor_mul(ob[:], ps[:], rb[:])
                    nc.gpsimd.tensor_scalar_add(
                        ob[:], ob[:], c0_t[:, et : et + 1]
                    )
                    nc.sync.dma_start(outT[et * 128 : (et + 1) * 128, :], ob[:])
                else:
                    # last tile: column halves pipeline DVE against DMA
                    for hh in range(2):
                        obh = posb.tile([128, LQ // 2], F32, tag=f"osbh{hh}")
                        sl = slice(hh * (LQ // 2), (hh + 1) * (LQ // 2))
                        nc.vector.tensor_mul(obh[:], ps[:, sl], rb[:, sl])
                        nc.gpsimd.tensor_scalar_add(
                            obh[:], obh[:], c0_t[:, et : et + 1]
                        )
                        nc.sync.dma_start(
                            outT[et * 128 : (et + 1) * 128, sl], obh[:]
                        )
            es_out.close()
            es_exp.close()
            es_wvo.close()
            es_pt.close()
            es_ppd.close()

        for rnd in range(n_rounds):
            emit_round(rnd)
        es_stats.close()

    split_multi_waits(nc)
    return nc


_PROGRAM = None


def _get_program():
    global _PROGRAM
    if _PROGRAM is None:
        _PROGRAM = build_program()
    return _PROGRAM


def _split_fp8(x):
    hi = x.astype(NP_FP8)
    lo = (x - hi.astype(np.float32)).astype(NP_FP8)
    return hi, lo


def build_in_maps(inputs):
    Q = np.asarray(inputs["Q"], dtype=np.float32)
    K = np.asarray(inputs["K"], dtype=np.float32)
    V = np.asarray(inputs["V"], dtype=np.float32)
    Wk = np.asarray(inputs["Wk"], dtype=np.float32)
    Wv = np.asarray(inputs["Wv"], dtype=np.float32)
    Wo = np.asarray(inputs["Wo"], dtype=np.float32)
    bv = np.asarray(inputs["bv"], dtype=np.float32)
    bo = np.asarray(inputs["bo"], dtype=np.float32)
    # bk drops out: it shifts every logit of a softmax row by the same
    # constant.

    Wvo = Wo @ Wv                                   # [D1, D2]
    c0 = Wo @ bv + bo                               # [D1]
    c0B_h = np.ascontiguousarray(c0.reshape(NT1, 128).T).astype(np.float32)

    # Wk [D1, D2] -> [128, ND1, 2, D2]: rows d1 = c*256 + j*128 + p
    WKH_h, WKL_h = (
        np.ascontiguousarray(w.reshape(ND1, 2, 128, D2).transpose(2, 0, 1, 3))
        for w in _split_fp8(32.0 * Wk)
    )
    # WvoT [D2, D1] -> [128, ND2, 2, D1]
    WVH_h, WVL_h = (
        np.ascontiguousarray(w.reshape(ND2, 2, 128, D1).transpose(2, 0, 1, 3))
        for w in _split_fp8(np.ascontiguousarray(64.0 * Wvo.T))
    )
    # KT [D2, LK] -> [128, NKB, ND2, 2, 512]:
    #   KT[256f + 128j + p, 512n + k] -> [p, n, f, j, k]
    KT_h = [
        tuple(
            np.ascontiguousarray(
                w.reshape(ND2, 2, 128, NKB, 512).transpose(2, 3, 0, 1, 4)
            )
            for w in _split_fp8(np.ascontiguousarray(K[b].T))
        )
        for b in range(B)
    ]
    # V [LK, D2] -> [128, 2, NDK, 2, H2]:
    #   V[256kt + 128j + p, 640h + d] -> [p, h, kt, j, d]
    VN_h = [
        tuple(
            np.ascontiguousarray(
                w.reshape(NDK, 2, 128, 2, H2).transpose(2, 3, 0, 1, 4)
            )
            for w in _split_fp8(V[b])
        )
        for b in range(B)
    ]

    in_maps = []
    for core in range(N_CORES):
        b, h = divmod(core, 2)
        QT = np.ascontiguousarray(Q[b, h * LQ : (h + 1) * LQ, :].T)
        # QT [D1, LQ] -> [128, ND1, 2, LQ]
        QH_h, QL_h = (
            np.ascontiguousarray(
                w.reshape(ND1, 2, 128, LQ).transpose(2, 0, 1, 3)
            )
            for w in _split_fp8(QT)
        )
        in_maps.append(
            {
                "QH": QH_h,
                "QL": QL_h,
                "WKH": WKH_h,
                "WKL": WKL_h,
                "KH": KT_h[b][0],
                "KL": KT_h[b][1],
                "VH": VN_h[b][0],
                "VL": VN_h[b][1],
                "WVH": WVH_h,
                "WVL": WVL_h,
                "c0B": c0B_h,
            }
        )
    return in_maps


def assemble_output(results):
    out = np.empty((B, 2 * LQ, D1), dtype=np.float32)
    for c in range(N_CORES):
        b, h = divmod(c, 2)
        out[b, h * LQ : (h + 1) * LQ, :] = results[c]["outT"].T
    return out


def kernel(Q, K, V, Wk, bk, Wv, bv, Wo, bo):
    inputs = dict(Q=Q, K=K, V=V, Wk=Wk, bk=bk, Wv=Wv, bv=bv, Wo=Wo, bo=bo)
    nc = _get_program()
    in_maps = build_in_maps(inputs)
    res = run_bass_kernel_spmd(nc, in_maps, list(range(N_CORES)))
    return assemble_output(res.results)
